# revision 21
# baseline (speedup 1.0000x reference)
"""Non-local block (embedded-dot-product, softmax-free) Trainium2 kernel, v2.

Reference computation:
    theta/phi/g = 1x1 conv projections of x [B,C,H,W] -> [B,Ci,N]
    f = (theta^T phi)/N  [B,N,N];  y = f @ g^T  [B,N,Ci]
    out = BN(W(y)) + x

Algebra (no softmax => matmul associativity):
    S_ref = phi_x @ g_x / N                  [Ci,Ci]   (g pre-scaled by 1/N)
    M3    = Weff @ S_ref^T                   [C,Ci]    (Weff = w_w * bn_inv)
    out   = (M3 Tw) X + (M3 tb + D) 1^T + X  where D folds W-bias + BN consts.
The theta projection is folded into the output matmul via WT = M3 Tw, so the
theta intermediate never materializes; the NxN affinity never materializes.

Optimizations over the fp32 v1 (HW-profile driven):
- bf16 HBM I/O: host casts x down / result up; halves DMA bytes and the
  2e-2 gate leaves orders of magnitude of slack (measured rel err 4.7e-3).
- theta fold: WT = M3 Tw means the theta projection never materializes.
- residual fold: WT' = WT + I (identity added during the WT PSUM drain via
  a host-precomputed eye block) makes the tail matmul produce w_y + x
  directly, so the output drain is one per-partition bias apply on ACT/DVE
  and GPSIMD (Q7 tensor ops ~2.3 ns/elem) never gates anything.
- paired pg PSUM groups (two 256-col matmul groups per bank; start=True
  clears has_written bits but not finished groups' data) halve drain count.
- all x input as [128,2048] pieces on the scalar HWDGE ring in sample
  order (rings share SDMA bandwidth — splitting the critical stream across
  rings delays it); outputs stream as 2-chunk pieces on the sync ring so
  the last transfer exposure is ~1.3 us.
- sample-serial schedule: sample 0's output DMA overlaps sample 1's
  projections; ~20 dummy matmuls on a memset tile during the ~7 us NRT
  preamble pre-warm the PE HAM clock gate (cold PE runs at 1.2 GHz).

Sharding: data-parallel over batch, 2 samples per core on 8 cores.
"""

import numpy as np
import ml_dtypes

import concourse.bass as bass
import concourse.mybir as mybir
import concourse.tile as tile
from concourse.bass_utils import run_bass_kernel_spmd

F32 = mybir.dt.float32
BF16 = mybir.dt.bfloat16
NPBF16 = ml_dtypes.bfloat16
ADD = mybir.AluOpType.add
IDENT = mybir.ActivationFunctionType.Identity

B, C, N, CI = 16, 256, 4096, 128
NCORES = 8
BL = B // NCORES  # samples per core
EPS = 1e-5

NT = N // 128  # 32 spatial tiles
NPAIR = NT // 2  # 16 pg pairs
NF = N // 512  # 8 output chunks per channel-half
PIECE = 2048  # input DMA piece width


# This walrus build rejects any instruction encoding more than one sync-wait.
# Tile freely emits multi-wait instructions, so post-process the finished
# module: excess waits move onto same-engine NOPs inserted just before the
# instruction (the engine blocks on each in turn — semantically identical).
def _split_multiwait(nc):
    n_split = 0
    for fn in nc.m.functions:
        for bb in fn.blocks:
            out = []
            for inst in bb.instructions:
                si = getattr(inst, "sync_info", None)
                if si is not None and si.on_wait and len(si.on_wait) > 1:
                    waits = list(si.on_wait)
                    si.on_wait = [waits[-1]]
                    for i, w in enumerate(waits[:-1]):
                        out.append(
                            mybir.InstNoOp(
                                name=f"{inst.name}-sw{i}",
                                engine=inst.engine,
                                sync_info=mybir.SyncInfo(on_wait=[w], on_update=[]),
                                bass_nofuse=True,
                            )
                        )
                    n_split += 1
                out.append(inst)
            bb.instructions[:] = out
    return n_split


_NC = {}


def build_nc(repeat=1, **opts):
    """Build the per-core Bass module. repeat>1 wraps the body in a device-side
    For_i loop (same data recomputed; used only for wall-clock slope timing)."""
    key = (repeat, tuple(sorted(opts.items())))
    if key in _NC:
        return _NC[key]
    s_lag = opts.get("s_lag", 5)  # S matmul trails pg copies by this many pairs
    # per-pair pg drain engines: F=DVE-fused add, D=ACT copy + DVE add,
    # G=ACT copy + GPSIMD add (GP is slow; keep its share small)
    pg_cycle = opts.get("pg_cycle", "FDDGFDDG")
    stt_cycle = opts.get("stt_cycle", "AV")  # output drain: ACT / DVE
    warm_mms = opts.get("warm_mms", 20)  # HAM pre-warm matmuls during preamble
    nc = bass.Bass()

    x_d = nc.declare_dram_parameter("x", [BL, C, N], BF16, isOutput=False)
    pgw_d = nc.declare_dram_parameter("pgw", [C, 2 * CI], BF16, isOutput=False)
    pgb2_d = nc.declare_dram_parameter("pgb2", [128, 512], BF16, isOutput=False)
    twr_d = nc.declare_dram_parameter("twr", [CI, C], BF16, isOutput=False)
    tbw_d = nc.declare_dram_parameter("tbw", [CI, 1], BF16, isOutput=False)
    ww_d = nc.declare_dram_parameter("ww", [CI, C], BF16, isOutput=False)
    wd_d = nc.declare_dram_parameter("wd", [128, 2], F32, isOutput=False)
    eye_d = nc.declare_dram_parameter("eye", [128, 2 * C], BF16, isOutput=False)
    out_d = nc.declare_dram_parameter("out", [BL, C, N], BF16, isOutput=True)

    with tile.TileContext(nc) as tc:
        with (
            tc.tile_pool(name="consts", bufs=1) as cpool,
            tc.tile_pool(name="xf", bufs=2 * BL) as xfp,
            tc.tile_pool(name="pg", bufs=2 * (s_lag + 2)) as pgp,
            tc.tile_pool(name="ptmp", bufs=3) as ptp,
            tc.tile_pool(name="small", bufs=2) as smp,
            tc.tile_pool(name="ob", bufs=3) as obp,
            tc.tile_pool(name="pgps", bufs=2, space="PSUM") as pgps,
            tc.tile_pool(name="sps", bufs=1, space="PSUM") as sps,
            tc.tile_pool(name="ps512", bufs=3, space="PSUM") as ps512,
        ):
            # ---- constants into SBUF ----
            # pg weights + bias ride the sync ring (input x owns the scalar
            # ring); phase-B constants go via gpsimd SWDGE.
            pgw_sb = cpool.tile([128, 2, 2 * CI], BF16)
            for k in range(2):
                nc.sync.dma_start(pgw_sb[:, k, :], pgw_d[k * 128 : (k + 1) * 128, :])
            pgb2_sb = cpool.tile([128, 512], BF16)
            nc.sync.dma_start(pgb2_sb[:], pgb2_d[:])
            twr_sb = cpool.tile([128, C], BF16)
            nc.gpsimd.dma_start(twr_sb[:], twr_d[:])
            tbw_sb = cpool.tile([128, 1], BF16)
            nc.gpsimd.dma_start(tbw_sb[:], tbw_d[:])
            ww_sb = cpool.tile([128, C], BF16)
            nc.gpsimd.dma_start(ww_sb[:], ww_d[:])
            wd_sb = cpool.tile([128, 2], F32)
            nc.gpsimd.dma_start(wd_sb[:], wd_d[:])
            eye_sb = cpool.tile([128, 2, C], BF16)
            nc.gpsimd.dma_start(eye_sb[:], eye_d[:])
            # HAM pre-warm source (no DMA dependency): the PE runs dummy
            # matmuls during the NRT preamble / input lead-in so the clock
            # gate opens (K=8/8) before the first real projection matmul
            wsrc = cpool.tile([128, 256], BF16)
            nc.vector.memset(wsrc[:], 0.125)

            def _body():
                # ---- all input DMAs issue first (sync ring) ----
                xfs = []
                for b in range(BL):
                    xf = [
                        xfp.tile([128, N], BF16, name="xf_t", uniquify=True)
                        for h in range(2)
                    ]
                    xfs.append(xf)
                for _ in range(warm_mms):
                    warm_ps = ps512.tile([128, 512], F32, name="w_ps",
                                         uniquify=True, bufs=3)
                    nc.tensor.matmul(
                        warm_ps[:, :256], lhsT=wsrc[:, :128], rhs=wsrc[:],
                        start=True, stop=True,
                    )
                for b in range(BL):
                    for p in range(N // PIECE):
                        for h in range(2):
                            nc.scalar.dma_start(
                                xfs[b][h][:, p * PIECE : (p + 1) * PIECE],
                                x_d[b, h * 128 : (h + 1) * 128,
                                    p * PIECE : (p + 1) * PIECE],
                            )

                # per-sample state carried between stages
                st = [dict() for _ in range(BL)]

                def emit_pgS(b, inject=None, inject_at=3):
                    """phi/g projections + S accumulation for sample b."""
                    xf = xfs[b]
                    s_ps = sps.tile([128, CI], F32, name="s_ps", uniquify=True)
                    st[b]["s_ps"] = s_ps
                    pgt = []

                    def s_mms(pr, last):
                        pg_t = pgt[pr]
                        for e in range(2):
                            t = 2 * pr + e
                            nc.tensor.matmul(
                                s_ps[:],
                                lhsT=pg_t[:, e * 256 + 128 : e * 256 + 256],
                                rhs=pg_t[:, e * 256 : e * 256 + 128],
                                start=(t == 0),
                                stop=(last and e == 1),
                            )

                    for pr in range(NPAIR):
                        pg_ps = pgps.tile([128, 512], F32, name="pg_ps",
                                          uniquify=True)
                        for e in range(2):
                            t = 2 * pr + e
                            dst = pg_ps[:, e * 256 : (e + 1) * 256]
                            for k in range(2):
                                nc.tensor.matmul(
                                    dst,
                                    lhsT=xf[k][:, t * 128 : (t + 1) * 128],
                                    rhs=pgw_sb[:, k, :],
                                    start=(k == 0),
                                    stop=(k == 1),
                                )
                        pg_t = pgp.tile([128, 512], BF16, name="pg_t", uniquify=True)
                        mode = pg_cycle[pr % len(pg_cycle)]
                        if mode == "F":
                            nc.vector.tensor_add(pg_t[:], pg_ps[:], pgb2_sb[:])
                        else:
                            ptmp = ptp.tile([128, 512], BF16, name="ptmp",
                                            uniquify=True)
                            nc.scalar.copy(ptmp[:], pg_ps[:])
                            eng = nc.vector if mode == "D" else nc.gpsimd
                            eng.tensor_add(pg_t[:], ptmp[:], pgb2_sb[:])
                        pgt.append(pg_t)
                        if pr == inject_at and inject is not None:
                            inject()
                        if pr >= s_lag:
                            s_mms(pr - s_lag, last=False)
                    for pr in range(NPAIR - s_lag, NPAIR):
                        s_mms(pr, last=(pr == NPAIR - 1))

                def emit_B1(b):
                    """S -> s_sb -> M3^T matmul."""
                    s_sb = smp.tile([128, CI], BF16, name="s_sb", uniquify=True, bufs=2)
                    nc.scalar.copy(s_sb[:], st[b]["s_ps"][:])
                    m_ps = ps512.tile([128, 512], F32, name="m_ps", uniquify=True, bufs=1)
                    nc.tensor.matmul(
                        m_ps[:, : 2 * CI], lhsT=s_sb[:], rhs=ww_sb[:],
                        start=True, stop=True,
                    )
                    m_sb = smp.tile([128, 2 * CI], BF16, name="m_sb", uniquify=True, bufs=2)
                    nc.vector.tensor_copy(m_sb[:], m_ps[:, : 2 * CI])
                    st[b]["m_sb"] = m_sb

                def emit_B2(b):
                    """M3^T -> WT^T ([C,C] fold of theta into W) and b2 bias."""
                    m_sb = st[b]["m_sb"]
                    wtt_sb = smp.tile([128, 2, C], BF16, name="wtt", uniquify=True, bufs=2)
                    b2d = smp.tile([128, 2], F32, name="b2d", uniquify=True, bufs=2)
                    for h in range(2):
                        w_ps = ps512.tile([128, 512], F32, name="wtt_ps",
                                          uniquify=True, bufs=1)
                        nc.tensor.matmul(
                            w_ps[:, :C],
                            lhsT=twr_sb[:, h * 128 : (h + 1) * 128],
                            rhs=m_sb[:],
                            start=True, stop=True,
                        )
                        nc.tensor.matmul(
                            w_ps[:, 448:449],
                            lhsT=m_sb[:, h * 128 : (h + 1) * 128],
                            rhs=tbw_sb[:],
                            start=True, stop=True,
                        )
                        # the copy folds in the identity block: WT' = M3 Tw + I
                        # makes the tail matmul compute w_y + x directly
                        nc.vector.tensor_add(
                            wtt_sb[:, h, :], w_ps[:, :C], eye_sb[:, h, :]
                        )
                        nc.vector.tensor_add(
                            b2d[:, h : h + 1], w_ps[:, 448:449], wd_sb[:, h : h + 1]
                        )
                    st[b]["wtt_sb"] = wtt_sb
                    st[b]["b2d"] = b2d

                def emit_tail(b, inject=None, inject_at=1):
                    """out = WT X + (b2+D) + X, staged per channel-half, 1MB DMA."""
                    xf = xfs[b]
                    wtt_sb = st[b]["wtt_sb"]
                    b2d = st[b]["b2d"]
                    for q in range(2):
                        ob = obp.tile([128, N], BF16, name="ob_t", uniquify=True)
                        for f in range(NF):
                            w_ps = ps512.tile([128, 512], F32, name="w_ps",
                                              uniquify=True, bufs=3)
                            for k in range(2):
                                nc.tensor.matmul(
                                    w_ps[:],
                                    lhsT=wtt_sb[:, k, q * 128 : (q + 1) * 128],
                                    rhs=xf[k][:, f * 512 : (f + 1) * 512],
                                    start=(k == 0),
                                    stop=(k == 1),
                                )
                            cols = slice(f * 512, (f + 1) * 512)
                            # residual already inside w_ps (WT' = WT + I);
                            # just apply the per-partition bias while draining
                            if stt_cycle[(q * NF + f) % len(stt_cycle)] == "A":
                                nc.scalar.activation(
                                    ob[:, cols], w_ps[:], IDENT,
                                    bias=b2d[:, q : q + 1],
                                )
                            else:
                                nc.vector.tensor_scalar_add(
                                    ob[:, cols], w_ps[:], b2d[:, q : q + 1]
                                )
                            if f == inject_at and q == 0 and inject is not None:
                                inject()
                        nc.sync.dma_start(
                            out_d[b, q * 128 : (q + 1) * 128, :], ob[:]
                        )

                # ---- schedule: sample-serial — sample 0's output DMA
                # overlaps sample 1's projections; input is dual-ring so
                # sample 1's x lands long before pgS(1) needs it ----
                for b in range(BL):
                    emit_pgS(b)
                    emit_B1(b)
                    emit_B2(b)
                    emit_tail(b)

            if repeat == 1:
                _body()
            else:
                with tc.For_i(0, repeat, 1):
                    _body()

    if not opts.get("nosplit", False):
        _split_multiwait(nc)
    _NC[key] = nc
    return nc


def _host_consts(inputs):
    """Fold biases/BN on the host; returns per-core constant input arrays."""
    g_w = np.asarray(inputs["g_w"], np.float32)
    g_b = np.asarray(inputs["g_b"], np.float32)
    theta_w = np.asarray(inputs["theta_w"], np.float32)
    theta_b = np.asarray(inputs["theta_b"], np.float32)
    phi_w = np.asarray(inputs["phi_w"], np.float32)
    phi_b = np.asarray(inputs["phi_b"], np.float32)
    w_w = np.asarray(inputs["w_w"], np.float32)
    w_b = np.asarray(inputs["w_b"], np.float32)
    bn_gamma = np.asarray(inputs["bn_gamma"], np.float32)
    bn_beta = np.asarray(inputs["bn_beta"], np.float32)
    bn_mean = np.asarray(inputs["bn_mean"], np.float32)
    bn_var = np.asarray(inputs["bn_var"], np.float32)

    inv = bn_gamma / np.sqrt(bn_var + EPS)  # [C]
    # [C, 2Ci]: cols 0:Ci = phi_w^T, Ci:2Ci = g_w^T / N
    pgw = np.ascontiguousarray(
        np.concatenate([phi_w.T, g_w.T / float(N)], axis=1)
    ).astype(NPBF16)
    pgb = np.concatenate([phi_b, g_b / float(N)])  # [2Ci]
    pgb2 = np.tile(pgb[None, :], (128, 2)).astype(NPBF16)  # [128, 512]
    twr = np.ascontiguousarray(theta_w).astype(NPBF16)  # [Ci, C]
    tbw = theta_b.reshape(CI, 1).astype(NPBF16)
    ww = np.ascontiguousarray((w_w * inv[:, None]).T).astype(NPBF16)  # [Ci, C]
    d = (w_b * inv + bn_beta - bn_mean * inv).astype(np.float32)  # [C]
    wd = np.ascontiguousarray(d.reshape(2, 128).T)  # [128, 2]
    # identity blocks added to WT^T during the PSUM drain: row r of half h
    # carries channel c = 128h + r, so the 1.0 sits at column 128h + r
    eye = np.zeros((128, 2, C), np.float32)
    for h in range(2):
        eye[np.arange(128), h, h * 128 + np.arange(128)] = 1.0
    eye = np.ascontiguousarray(eye.reshape(128, 2 * C)).astype(NPBF16)
    return dict(pgw=pgw, pgb2=pgb2, twr=twr, tbw=tbw, ww=ww, wd=wd, eye=eye)


def _in_maps(inputs):
    x = np.asarray(inputs["x"], np.float32).reshape(B, C, N).astype(NPBF16)
    consts = _host_consts(inputs)
    return [
        {"x": np.ascontiguousarray(x[i * BL : (i + 1) * BL]), **consts}
        for i in range(NCORES)
    ]


def _gather(results):
    out = np.concatenate([r["out"] for r in results], axis=0)
    return out.astype(np.float32).reshape(B, C, 64, 64)


def kernel(**inputs):
    in_maps = _in_maps(inputs)
    nc = build_nc()
    res = run_bass_kernel_spmd(nc, in_maps, core_ids=list(range(NCORES)))
    return _gather(res.results)
